# revision 35
# baseline (speedup 1.0000x reference)
"""Causal multi-head attention (B=4, T=2048, D=1024, 16 heads x 64) on 8 trn2 cores.

Sharding: tensor-parallel over heads, 2 heads per core. Every core receives the
full activations x (pre-transposed on host to [B, D, T], cast bf16) plus its 2
heads' worth of W_Q/W_K/W_V pre-arranged to [128, 8*128] bf16; it computes full
causal attention for its heads and writes out z^T plus the softmax denominator
row per head ([B, 2, 65, T] bf16). The host normalizes and re-lays-out.

Device kernel layout (per core):
  - projections produce Q^T/K^T/V^T head-major [128(2h), T]; V^T is
    PE-transposed per 128-block into the AV stationary (v_aug).
  - scores computed transposed S^T[kt, qt] so the two heads run as concurrent
    PE row-tiles (K=64 at partition bases 0/64) into the two halves of one
    [128, 1024] PSUM pair.
  - exp is LOAD-BALANCED between the Scalar engine (true Exp) and the Vector
    engine (Schraudolph bit-trick: bf16(exp(s/8)) ~= int16(A*s + B) bit
    pattern, one tensor_scalar per chunk). PSUM->SBUF casts/copies are also
    routed through the same balancer.
  - causal mask applied post-exp with gpsimd affine_select (idle engine);
    diagonal chunks narrowed to their valid column range.
  - softmax denominator comes free from an all-ones column in v_aug,
    accumulated in the same f32 PSUM as z^T; AV is software-pipelined a few
    chunks behind the scores.
  - the NEXT batch's projection gemms + V-transposes are interleaved into the
    current batch's attention chunk loop as PE filler, so the exp engines and
    the PE stay concurrently busy across the whole kernel.
  - a short burst of zero matmuls at kernel start warms the PE HAM clock gate
    (1.2 -> 2.4 GHz) while the first x chunks stream in.
"""

import os
import sys

for _p in ("/opt/trn_rl_repo", "/root/.axon_site/_ro/trn_rl_repo"):
    if os.path.isdir(_p) and _p not in sys.path:
        sys.path.insert(0, _p)

import ml_dtypes
import numpy as np

import concourse.bass as bass
import concourse.mybir as mybir
import concourse.tile as tile
from concourse.masks import make_identity
from concourse import bacc
from concourse.bass import ds
from concourse.bass_utils import run_bass_kernel_spmd

B, T, D = 4, 2048, 1024
NH, DH = 16, 64
NCORES = 8
HPC = NH // NCORES          # heads per core = 2
H2 = HPC * DH               # packed per-core head dim = 128
P = 128
QT = 512                    # query-tile width (psum bank limit for f32 out)
NQ4 = T // QT               # 4 query tiles
NCH = T // P                # 16 key chunks
KD = D // P                 # 8 contraction chunks
F32 = mybir.dt.float32
BF16 = mybir.dt.bfloat16
I16 = mybir.dt.int16
SCALE = 1.0 / np.sqrt(DH)   # 0.125
LOG2E = 1.4426950408889634
SCH_A = 128.0 * LOG2E * SCALE           # bf16-bit-space slope
SCH_B = 16256.0 - 0.0579 * 128.0        # 127<<7 minus mean-centering term


def _build(nc, tc, xT_d, x0p_d, w_d, out_d):
    from contextlib import ExitStack

    AF = mybir.ActivationFunctionType
    OP = mybir.AluOpType
    MPB = QT // P  # 128-blocks per query tile = 4

    with ExitStack() as ctx:
        ep = ctx.enter_context
        const = ep(tc.tile_pool(name="const", bufs=1))
        xt_pool = ep(tc.tile_pool(name="xt", bufs=2 * KD + 1))
        qk_pool = ep(tc.tile_pool(name="qk", bufs=4))
        vt_pool = ep(tc.tile_pool(name="vt", bufs=3))
        vaug_pool = ep(tc.tile_pool(name="vaug", bufs=2))
        p_pool = ep(tc.tile_pool(name="pp", bufs=9))
        zt_pool = ep(tc.tile_pool(name="zt", bufs=2))
        ps_acc = ep(tc.tile_pool(name="ps_acc", bufs=2, space="PSUM"))
        ps_s = ep(tc.tile_pool(name="ps_s", bufs=2, space="PSUM"))
        ps_z = ep(tc.tile_pool(name="ps_z", bufs=2, space="PSUM"))

        # ---- engine balancer: assigns exp chunks and psum->sbuf copies to
        # whichever of Scalar (1.2 GHz, +352cyc/inst) / Vector (1.92 GHz)
        # currently has less queued work.
        bal = {"act": 0.0, "dve": 0.0}

        def pick(cost_act, cost_dve):
            if bal["act"] + cost_act <= bal["dve"] + cost_dve:
                bal["act"] += cost_act
                return "act"
            bal["dve"] += cost_dve
            return "dve"

        def exp_ranges(pe_t, ss_t, ranges):
            # measured: ACT ~ N*1.0 + 300ns/inst; DVE 2-op TS ~ N*1.39 + 60
            ncols = sum(w for _, w in ranges)
            ca = 1.0 * ncols + 300 * len(ranges)
            cd = 1.39 * ncols + 60 * len(ranges)
            eng = pick(ca, cd)
            for lo, w in ranges:
                if eng == "act":
                    nc.scalar.activation(
                        pe_t[:, lo : lo + w], ss_t[:, lo : lo + w],
                        AF.Exp, scale=float(SCALE),
                    )
                else:
                    nc.vector.tensor_scalar(
                        pe_t[:, lo : lo + w].bitcast(I16),
                        ss_t[:, lo : lo + w],
                        float(SCH_A), float(SCH_B),
                        OP.mult, OP.add,
                    )

        def dve_copy(dst, src, ncols):
            bal["dve"] += 1.04 * ncols + 156
            nc.vector.tensor_copy(dst, src)

        def bal_copy(dst, src, ncols):
            ca = 1.0 * ncols + 300
            cd = 1.04 * ncols + 156
            if pick(ca, cd) == "act":
                nc.scalar.copy(dst, src)
            else:
                nc.vector.tensor_copy(dst, src)

        # Startup: batch 0's x arrives as [128, 512] pieces in t4-major order
        # so the first projection group is never starved. The pieces come
        # from x0p, a host-prechunked contiguous copy of batch 0's x^T:
        # slicing xT directly would make each piece a strided DRAM read
        # (1 KB useful per 4 KB row) — measured several us slower to land.
        xts_pool = ep(tc.tile_pool(name="xts", bufs=NQ4 * KD // 2))
        xch0p = [[None] * KD for _ in range(NQ4 // 2)]
        # Weight DMAs issue from the Scalar engine's hwdge: it is idle until
        # its first exp (~15us), and taking their ~1.7us of descriptor work
        # off the Sync sequencer lets the t4=0 x pieces issue earlier.
        w_sb = {}
        for name in ("wv", "wq", "wk"):
            t = const.tile([P, KD, H2], BF16, tag=name)
            nc.scalar.dma_start(t[:], w_d[name].rearrange("p (c h) -> p c h", c=KD))
            w_sb[name] = t
        for tp in range(NQ4 // 2):
            for k in range(KD):
                tt = xts_pool.tile([P, 2 * QT], BF16, tag="xts", name="xts")
                nc.sync.dma_start(tt[:], x0p_d[tp, k, :, :])
                xch0p[tp][k] = tt

        def xch0(t4, k):
            return xch0p[t4 // 2][k][:, ds((t4 % 2) * QT, QT)]

        # HAM warm-up: zero matmuls keep the PE busy through the ~13us
        # weight/x DMA window so the clock gate opens before real work.
        warm_sb = const.tile([P, QT], BF16, tag="warm")
        nc.gpsimd.memset(warm_sb[:], 0.0)
        warm_ps = ps_s.tile([P, 2 * QT], F32, tag="s", name="warm")
        for _ in range(14):
            nc.tensor.matmul(
                warm_ps[:, 0:QT], warm_sb[:, 0:P], warm_sb[:],
                start=True, stop=True, skip_group_check=True,
            )

        ident = const.tile([P, P], BF16, tag="ident")
        make_identity(nc, ident)

        # v_aug double buffers: [kt, chunk, 64 v-cols | ones col]. Only 65
        # stationary columns: the 63 zero columns of the old 128-wide layout
        # bought nothing (LDWEIGHTS time is column-count-independent) and
        # cost SBUF + memsets.
        VA = DH + 1   # AV stationary columns (64 v + ones)
        vaug = []
        for _bb in range(2):
            pair = []
            for h in range(HPC):
                v = vaug_pool.tile([P, NCH, VA], BF16, tag=f"v{h}")
                nc.gpsimd.memset(v[:, :, DH : DH + 1], 1.0)
                pair.append(v)
            vaug.append(pair)

        # ---- per-batch projection work, expressed as a generator of PE
        # micro-ops so it can be drained as filler inside the previous
        # batch's attention loop.
        qkt = {}   # b -> (qt_sb, kt_sb)

        def start_batch(b):
            """Issue x DMA for batch b (b>0) and allocate its qt/kt tiles."""
            if b == 0:
                xch = None
            else:
                xch = []
                for k in range(KD):
                    xt_t = xt_pool.tile([P, T], BF16, tag="xt", name="xt_t")
                    nc.sync.dma_start(xt_t[:], xT_d[b, ds(k * P, P), :])
                    xch.append(xt_t)
            qt_sb = qk_pool.tile([P, T], BF16, tag="qt")
            kt_sb = qk_pool.tile([P, T], BF16, tag="kt")
            qkt[b] = (qt_sb, kt_sb)
            return xch

        def proj_gen(b, xch):
            """Yield once per PE micro-op (matmul).

            Per t4 stage: wv gemm -> vt copy -> DMA-XBAR transposes of V^T
            into the AV stationary (off the PE entirely; the DMA engines are
            mostly idle and va isn't needed until this batch's attention) ->
            wq gemm -> wk gemm."""
            qt_sb, kt_sb = qkt[b]
            va = vaug[b % 2]

            def gemm(name, acc, t4):
                # yields KD-1 times; caller issues the psum->sbuf copy and
                # then the stage's closing yield, so every copy lands inside
                # its 24-op stage (stage-gated drains rely on this).
                for k in range(KD):
                    rhs = (
                        xch0(t4, k)
                        if b == 0
                        else xch[k][:, ds(t4 * QT, QT)]
                    )
                    nc.tensor.matmul(
                        acc[:],
                        w_sb[name][:, k, :],
                        rhs,
                        start=(k == 0),
                        stop=(k == KD - 1),
                    )
                    if k < KD - 1:
                        yield

            for t4 in range(NQ4):
                acc = ps_acc.tile([P, QT], F32, tag="acc", name="accv")
                yield from gemm("wv", acc, t4)
                vt_t = vt_pool.tile([P, QT], BF16, tag="vt", name="vt_t")
                bal_copy(vt_t[:], acc[:], QT)
                yield
                acc = ps_acc.tile([P, QT], F32, tag="acc", name="accq")
                yield from gemm("wq", acc, t4)
                bal_copy(qt_sb[:, ds(t4 * QT, QT)], acc[:], QT)
                yield
                for m in range(MPB):
                    j = t4 * MPB + m
                    pt = ps_acc.tile([P, P], BF16, tag="acc", name="pt")
                    nc.tensor.transpose(pt[:], vt_t[:, ds(m * P, P)], ident[:])
                    for h in range(HPC):
                        dve_copy(va[h][:, j, 0:DH], pt[:, ds(h * DH, DH)], DH)
                    yield
                acc = ps_acc.tile([P, QT], F32, tag="acc", name="acck")
                yield from gemm("wk", acc, t4)
                bal_copy(kt_sb[:, ds(t4 * QT, QT)], acc[:], QT)
                yield

        def drain(gen, n):
            if gen is None:
                return
            for _ in range(n):
                if next(gen, "done") == "done":
                    return

        # ---- main loop: software-pipelined attention(b) + projections.
        # Attention query-tile q4 only needs proj stage t4<=q4 (x DMA arrives
        # t4-major), so each batch's attention is stage-gated on its OWN
        # projection generator, which is drained partly as filler inside the
        # PREVIOUS batch's attention and partly inside its own early query
        # tiles. This keeps PE filler present in every batch - including the
        # last one - so the exp engines' latency never idles the PE long
        # enough for the HAM clock gate to re-throttle.
        SOPS = 3 * KD + MPB  # micro-ops per proj stage (28)
        start_batch(0)
        g_own = proj_gen(0, None)
        own_done = 0  # micro-ops of g_own drained so far

        for b in range(B):
            gnext = None
            next_done = 0

            qt_sb, kt_sb = qkt[b]
            va = vaug[b % 2]
            zt_sb = [
                zt_pool.tile([DH + 1, T], BF16, tag=f"z{h}", name=f"ztb{h}")
                for h in range(HPC)
            ]
            for q4 in range(NQ4):
                if q4 == 1 and b + 1 < B:
                    # issue the next batch's x DMA only now, so it doesn't
                    # contend with this batch's own t4-major stage pieces
                    xch_next = start_batch(b + 1)
                    gnext = proj_gen(b + 1, xch_next)
                # release this batch's proj stage t4=q4 before its query tile
                need = (q4 + 1) * SOPS
                if own_done < need:
                    drain(g_own, need - own_done)
                    own_done = need
                njs = (q4 + 1) * MPB
                pz = [
                    ps_z.tile([P, QT], F32, tag="z", name="pz") for _ in range(HPC)
                ]
                pend = []  # (j, c0, exp tile) awaiting the AV matmuls
                for j in range(njs):
                    rdiag = j - q4 * MPB  # >=0 on diagonal-overlap chunks
                    last = j == njs - 1
                    c0 = 0 if rdiag < 0 else rdiag * P
                    w_hi = (rdiag + 1) * P if rdiag >= 0 else 0
                    nw = QT - c0
                    ss = ps_s.tile([P, 2 * QT], F32, tag="s")
                    pe = p_pool.tile([P, 2 * QT], BF16, tag="p", name="pe")
                    for h in range(HPC):
                        hp = ds(h * DH, DH)
                        nc.tensor.matmul(
                            ss[:, h * QT + c0 : (h + 1) * QT],
                            kt_sb[hp, ds(j * P, P)],
                            qt_sb[hp, ds(q4 * QT + c0, nw)],
                            start=True,
                            stop=True,
                        )
                    if c0 == 0:
                        exp_ranges(pe, ss, [(0, 2 * QT)])
                    else:
                        exp_ranges(
                            pe, ss,
                            [(h * QT + c0, nw) for h in range(HPC)],
                        )
                    if rdiag >= 0:
                        # keep iff qt >= kt  <=>  (col - p - 128*rdiag) >= 0
                        for h in range(HPC):
                            nc.gpsimd.affine_select(
                                out=pe[:, h * QT + c0 : h * QT + w_hi],
                                in_=pe[:, h * QT + c0 : h * QT + w_hi],
                                compare_op=OP.is_ge,
                                fill=0.0,
                                base=c0 - rdiag * P,
                                pattern=[[1, w_hi - c0]],
                                channel_multiplier=-1,
                            )
                    # PE filler between this chunk's scores and the
                    # (possibly exp-waiting) AV: finish this batch's own proj
                    # first, then feed the next batch's at a steady rate,
                    # leaving its last stages for attention(b+1) to carry.
                    if own_done < NQ4 * SOPS:
                        take = min(3, NQ4 * SOPS - own_done)
                        drain(g_own, take)
                        own_done += take
                    elif gnext is not None and next_done < NQ4 * SOPS:
                        take = min(3, NQ4 * SOPS - next_done)
                        drain(gnext, take)
                        next_done += take
                    pend.append((j, c0, pe))
                    # software-pipeline: AV runs a few chunks behind scores
                    if len(pend) > 5 or last:
                        for jj, cc0, ppe in pend if last else [pend[0]]:
                            for h in range(HPC):
                                nc.tensor.matmul(
                                    pz[h][0:VA, cc0:QT],
                                    va[h][:, jj, :],
                                    ppe[:, h * QT + cc0 : (h + 1) * QT],
                                    start=(jj == 0),
                                    stop=(jj == njs - 1),
                                    skip_group_check=True,
                                )
                        pend = [] if last else pend[1:]

                for h in range(HPC):
                    bal_copy(
                        zt_sb[h][:, ds(q4 * QT, QT)], pz[h][0:VA, :], QT
                    )
                    # z^T (+ denominator row) raw; host divides+transposes
                    nc.sync.dma_start(
                        out_d[b, h, :, ds(q4 * QT, QT)],
                        zt_sb[h][:, ds(q4 * QT, QT)],
                    )
            # hand the next batch's partially-drained projections over; its
            # own attention finishes them via stage gates and filler.
            g_own = gnext
            own_done = next_done if gnext is not None else 0


def build_bass():
    nc = bacc.Bacc(None, target_bir_lowering=False)
    xT_d = nc.declare_dram_parameter("xT", [B, D, T], BF16, isOutput=False)
    x0p_d = nc.declare_dram_parameter(
        "x0p", [NQ4 // 2, KD, P, 2 * QT], BF16, isOutput=False
    )
    w_d = {
        name: nc.declare_dram_parameter(name, [P, KD * H2], BF16, isOutput=False)
        for name in ("wq", "wk", "wv")
    }
    out_d = nc.declare_dram_parameter(
        "out", [B, HPC, DH + 1, T], BF16, isOutput=True
    )
    with tile.TileContext(nc) as tc:
        _build(nc, tc, xT_d, x0p_d, w_d, out_d)
    nc.compile()
    return nc


_CACHE = {}


def _get_nc():
    if "nc" not in _CACHE:
        _CACHE["nc"] = build_bass()
    return _CACHE["nc"]


def make_in_maps(x, W_K, W_Q, W_V):
    x = np.asarray(x, dtype=np.float32)
    xT = np.ascontiguousarray(np.transpose(x, (0, 2, 1))).astype(ml_dtypes.bfloat16)
    # batch-0 x^T prechunked into contiguous [t4, k, 128, 512] pieces
    x0p = np.ascontiguousarray(
        xT[0].reshape(KD, P, NQ4 // 2, 2 * QT).transpose(2, 0, 1, 3)
    )
    in_maps = []
    for c in range(NCORES):
        sl = slice(c * HPC, (c + 1) * HPC)

        def wt(w):
            w = np.asarray(w, dtype=np.float32)
            wt_ = w[sl].reshape(H2, D).T  # [D, H2]
            wt_ = wt_.reshape(KD, P, H2).transpose(1, 0, 2).reshape(P, KD * H2)
            return np.ascontiguousarray(wt_).astype(ml_dtypes.bfloat16)

        in_maps.append(
            {"xT": xT, "x0p": x0p, "wq": wt(W_Q), "wk": wt(W_K), "wv": wt(W_V)}
        )
    return in_maps


def kernel(x, W_K, W_Q, W_V, _trace=False, _trace_kwargs=None):
    in_maps = make_in_maps(x, W_K, W_Q, W_V)
    res = run_bass_kernel_spmd(
        _get_nc(),
        in_maps,
        list(range(NCORES)),
        trace=_trace,
        **(_trace_kwargs or {}),
    )
    _CACHE["last_results"] = res
    outs = []
    for c in range(NCORES):
        # [B, HPC, DH+1, T] bf16: z^T rows + softmax denominator row
        zt = np.asarray(res.results[c]["out"]).astype(np.float32)
        z = zt[:, :, :DH, :] / zt[:, :, DH : DH + 1, :]
        outs.append(np.transpose(z, (0, 3, 1, 2)).reshape(B, T, H2))
    return np.concatenate(outs, axis=2)



# revision 36
# speedup vs baseline: 1.0122x; 1.0122x over previous
"""Causal multi-head attention (B=4, T=2048, D=1024, 16 heads x 64) on 8 trn2 cores.

Sharding: tensor-parallel over heads, 2 heads per core. Every core receives the
full activations x (pre-transposed on host to [B, D, T], cast bf16) plus its 2
heads' worth of W_Q/W_K/W_V pre-arranged to [128, 8*128] bf16; it computes full
causal attention for its heads and writes out z^T plus the softmax denominator
row per head ([B, 2, 65, T] bf16). The host normalizes and re-lays-out.

Device kernel layout (per core):
  - projections produce Q^T/K^T/V^T head-major [128(2h), T]; V^T is
    PE-transposed per 128-block into the AV stationary (v_aug).
  - scores computed transposed S^T[kt, qt] so the two heads run as concurrent
    PE row-tiles (K=64 at partition bases 0/64) into the two halves of one
    [128, 1024] PSUM pair.
  - exp is LOAD-BALANCED between the Scalar engine (true Exp) and the Vector
    engine (Schraudolph bit-trick: bf16(exp(s/8)) ~= int16(A*s + B) bit
    pattern, one tensor_scalar per chunk). PSUM->SBUF casts/copies are also
    routed through the same balancer.
  - causal mask applied post-exp with gpsimd affine_select (idle engine);
    diagonal chunks narrowed to their valid column range.
  - softmax denominator comes free from an all-ones column in v_aug,
    accumulated in the same f32 PSUM as z^T; AV is software-pipelined a few
    chunks behind the scores.
  - the NEXT batch's projection gemms + V-transposes are interleaved into the
    current batch's attention chunk loop as PE filler, so the exp engines and
    the PE stay concurrently busy across the whole kernel.
  - a short burst of zero matmuls at kernel start warms the PE HAM clock gate
    (1.2 -> 2.4 GHz) while the first x chunks stream in.
"""

import os
import sys

for _p in ("/opt/trn_rl_repo", "/root/.axon_site/_ro/trn_rl_repo"):
    if os.path.isdir(_p) and _p not in sys.path:
        sys.path.insert(0, _p)

import ml_dtypes
import numpy as np

import concourse.bass as bass
import concourse.mybir as mybir
import concourse.tile as tile
from concourse.masks import make_identity
from concourse import bacc
from concourse.bass import ds
from concourse.bass_utils import run_bass_kernel_spmd

B, T, D = 4, 2048, 1024
NH, DH = 16, 64
NCORES = 8
HPC = NH // NCORES          # heads per core = 2
H2 = HPC * DH               # packed per-core head dim = 128
P = 128
QT = 512                    # query-tile width (psum bank limit for f32 out)
NQ4 = T // QT               # 4 query tiles
NCH = T // P                # 16 key chunks
KD = D // P                 # 8 contraction chunks
F32 = mybir.dt.float32
BF16 = mybir.dt.bfloat16
I16 = mybir.dt.int16
SCALE = 1.0 / np.sqrt(DH)   # 0.125
LOG2E = 1.4426950408889634
SCH_A = 128.0 * LOG2E * SCALE           # bf16-bit-space slope
SCH_B = 16256.0 - 0.0579 * 128.0        # 127<<7 minus mean-centering term


def _build(nc, tc, xT_d, x0p_d, w_d, out_d):
    from contextlib import ExitStack

    AF = mybir.ActivationFunctionType
    OP = mybir.AluOpType
    MPB = QT // P  # 128-blocks per query tile = 4

    with ExitStack() as ctx:
        ep = ctx.enter_context
        const = ep(tc.tile_pool(name="const", bufs=1))
        xt_pool = ep(tc.tile_pool(name="xt", bufs=2 * KD + 1))
        qk_pool = ep(tc.tile_pool(name="qk", bufs=4))
        vt_pool = ep(tc.tile_pool(name="vt", bufs=3))
        vaug_pool = ep(tc.tile_pool(name="vaug", bufs=2))
        p_pool = ep(tc.tile_pool(name="pp", bufs=9))
        zt_pool = ep(tc.tile_pool(name="zt", bufs=2))
        ps_acc = ep(tc.tile_pool(name="ps_acc", bufs=2, space="PSUM"))
        ps_s = ep(tc.tile_pool(name="ps_s", bufs=2, space="PSUM"))
        ps_z = ep(tc.tile_pool(name="ps_z", bufs=2, space="PSUM"))

        # ---- engine balancer: assigns exp chunks and psum->sbuf copies to
        # whichever of Scalar (1.2 GHz, +352cyc/inst) / Vector (1.92 GHz)
        # currently has less queued work.
        bal = {"act": 0.0, "dve": 0.0}

        def pick(cost_act, cost_dve):
            if bal["act"] + cost_act <= bal["dve"] + cost_dve:
                bal["act"] += cost_act
                return "act"
            bal["dve"] += cost_dve
            return "dve"

        def exp_ranges(pe_t, ss_t, ranges):
            # measured: ACT ~ N*1.0 + 300ns/inst; DVE 2-op TS ~ N*1.39 + 60
            ncols = sum(w for _, w in ranges)
            ca = 1.0 * ncols + 300 * len(ranges)
            cd = 1.39 * ncols + 60 * len(ranges)
            eng = pick(ca, cd)
            for lo, w in ranges:
                if eng == "act":
                    nc.scalar.activation(
                        pe_t[:, lo : lo + w], ss_t[:, lo : lo + w],
                        AF.Exp, scale=float(SCALE),
                    )
                else:
                    nc.vector.tensor_scalar(
                        pe_t[:, lo : lo + w].bitcast(I16),
                        ss_t[:, lo : lo + w],
                        float(SCH_A), float(SCH_B),
                        OP.mult, OP.add,
                    )

        def dve_copy(dst, src, ncols):
            bal["dve"] += 1.04 * ncols + 156
            nc.vector.tensor_copy(dst, src)

        def bal_copy(dst, src, ncols):
            ca = 1.0 * ncols + 300
            cd = 1.04 * ncols + 156
            if pick(ca, cd) == "act":
                nc.scalar.copy(dst, src)
            else:
                nc.vector.tensor_copy(dst, src)

        # Startup: batch 0's x arrives as [128, 512] pieces in t4-major order
        # so the first projection group is never starved. The pieces come
        # from x0p, a host-prechunked contiguous copy of batch 0's x^T:
        # slicing xT directly would make each piece a strided DRAM read
        # (1 KB useful per 4 KB row) — measured several us slower to land.
        xts_pool = ep(tc.tile_pool(name="xts", bufs=NQ4 * KD))
        xch0p = [[None] * KD for _ in range(NQ4)]
        # Weight DMAs issue from the Scalar engine's hwdge: it is idle until
        # its first exp (~15us), and taking their ~1.7us of descriptor work
        # off the Sync sequencer lets the t4=0 x pieces issue earlier.
        w_sb = {}
        for name in ("wv", "wq", "wk"):
            t = const.tile([P, KD, H2], BF16, tag=name)
            nc.scalar.dma_start(t[:], w_d[name].rearrange("p (c h) -> p c h", c=KD))
            w_sb[name] = t
        for t4 in range(NQ4):
            for k in range(KD):
                tt = xts_pool.tile([P, QT], BF16, tag="xts", name="xts")
                nc.sync.dma_start(tt[:], x0p_d[t4, k, :, :])
                xch0p[t4][k] = tt

        def xch0(t4, k):
            return xch0p[t4][k][:]

        # HAM warm-up: zero matmuls keep the PE busy through the ~13us
        # weight/x DMA window so the clock gate opens before real work.
        warm_sb = const.tile([P, QT], BF16, tag="warm")
        nc.gpsimd.memset(warm_sb[:], 0.0)
        warm_ps = ps_s.tile([P, 2 * QT], F32, tag="s", name="warm")
        for _ in range(14):
            nc.tensor.matmul(
                warm_ps[:, 0:QT], warm_sb[:, 0:P], warm_sb[:],
                start=True, stop=True, skip_group_check=True,
            )

        ident = const.tile([P, P], BF16, tag="ident")
        make_identity(nc, ident)

        # v_aug double buffers: [kt, chunk, 64 v-cols | ones col]. Only 65
        # stationary columns: the 63 zero columns of the old 128-wide layout
        # bought nothing (LDWEIGHTS time is column-count-independent) and
        # cost SBUF + memsets.
        VA = DH + 1   # AV stationary columns (64 v + ones)
        vaug = []
        for _bb in range(2):
            pair = []
            for h in range(HPC):
                v = vaug_pool.tile([P, NCH, VA], BF16, tag=f"v{h}")
                nc.gpsimd.memset(v[:, :, DH : DH + 1], 1.0)
                pair.append(v)
            vaug.append(pair)

        # ---- per-batch projection work, expressed as a generator of PE
        # micro-ops so it can be drained as filler inside the previous
        # batch's attention loop.
        qkt = {}   # b -> (qt_sb, kt_sb)

        def start_batch(b):
            """Issue x DMA for batch b (b>0) and allocate its qt/kt tiles."""
            if b == 0:
                xch = None
            else:
                xch = []
                for k in range(KD):
                    xt_t = xt_pool.tile([P, T], BF16, tag="xt", name="xt_t")
                    nc.sync.dma_start(xt_t[:], xT_d[b, ds(k * P, P), :])
                    xch.append(xt_t)
            qt_sb = qk_pool.tile([P, T], BF16, tag="qt")
            kt_sb = qk_pool.tile([P, T], BF16, tag="kt")
            qkt[b] = (qt_sb, kt_sb)
            return xch

        def proj_gen(b, xch):
            """Yield once per PE micro-op (matmul).

            Per t4 stage: wv gemm -> vt copy -> DMA-XBAR transposes of V^T
            into the AV stationary (off the PE entirely; the DMA engines are
            mostly idle and va isn't needed until this batch's attention) ->
            wq gemm -> wk gemm."""
            qt_sb, kt_sb = qkt[b]
            va = vaug[b % 2]

            def gemm(name, acc, t4):
                # yields KD-1 times; caller issues the psum->sbuf copy and
                # then the stage's closing yield, so every copy lands inside
                # its 24-op stage (stage-gated drains rely on this).
                for k in range(KD):
                    rhs = (
                        xch0(t4, k)
                        if b == 0
                        else xch[k][:, ds(t4 * QT, QT)]
                    )
                    nc.tensor.matmul(
                        acc[:],
                        w_sb[name][:, k, :],
                        rhs,
                        start=(k == 0),
                        stop=(k == KD - 1),
                    )
                    if k < KD - 1:
                        yield

            for t4 in range(NQ4):
                acc = ps_acc.tile([P, QT], F32, tag="acc", name="accv")
                yield from gemm("wv", acc, t4)
                vt_t = vt_pool.tile([P, QT], BF16, tag="vt", name="vt_t")
                bal_copy(vt_t[:], acc[:], QT)
                yield
                acc = ps_acc.tile([P, QT], F32, tag="acc", name="accq")
                yield from gemm("wq", acc, t4)
                bal_copy(qt_sb[:, ds(t4 * QT, QT)], acc[:], QT)
                yield
                for m in range(MPB):
                    j = t4 * MPB + m
                    pt = ps_acc.tile([P, P], BF16, tag="acc", name="pt")
                    nc.tensor.transpose(pt[:], vt_t[:, ds(m * P, P)], ident[:])
                    for h in range(HPC):
                        dve_copy(va[h][:, j, 0:DH], pt[:, ds(h * DH, DH)], DH)
                    yield
                acc = ps_acc.tile([P, QT], F32, tag="acc", name="acck")
                yield from gemm("wk", acc, t4)
                bal_copy(kt_sb[:, ds(t4 * QT, QT)], acc[:], QT)
                yield

        def drain(gen, n):
            if gen is None:
                return
            for _ in range(n):
                if next(gen, "done") == "done":
                    return

        # ---- main loop: software-pipelined attention(b) + projections.
        # Attention query-tile q4 only needs proj stage t4<=q4 (x DMA arrives
        # t4-major), so each batch's attention is stage-gated on its OWN
        # projection generator, which is drained partly as filler inside the
        # PREVIOUS batch's attention and partly inside its own early query
        # tiles. This keeps PE filler present in every batch - including the
        # last one - so the exp engines' latency never idles the PE long
        # enough for the HAM clock gate to re-throttle.
        SOPS = 3 * KD + MPB  # micro-ops per proj stage (28)
        start_batch(0)
        g_own = proj_gen(0, None)
        own_done = 0  # micro-ops of g_own drained so far

        for b in range(B):
            gnext = None
            next_done = 0

            qt_sb, kt_sb = qkt[b]
            va = vaug[b % 2]
            zt_sb = [
                zt_pool.tile([DH + 1, T], BF16, tag=f"z{h}", name=f"ztb{h}")
                for h in range(HPC)
            ]
            for q4 in range(NQ4):
                if q4 == 1 and b + 1 < B:
                    # issue the next batch's x DMA only now, so it doesn't
                    # contend with this batch's own t4-major stage pieces
                    xch_next = start_batch(b + 1)
                    gnext = proj_gen(b + 1, xch_next)
                # release this batch's proj stage t4=q4 before its query tile
                need = (q4 + 1) * SOPS
                if own_done < need:
                    drain(g_own, need - own_done)
                    own_done = need
                njs = (q4 + 1) * MPB
                pz = [
                    ps_z.tile([P, QT], F32, tag="z", name="pz") for _ in range(HPC)
                ]
                pend = []  # (j, c0, exp tile) awaiting the AV matmuls
                for j in range(njs):
                    rdiag = j - q4 * MPB  # >=0 on diagonal-overlap chunks
                    last = j == njs - 1
                    c0 = 0 if rdiag < 0 else rdiag * P
                    w_hi = (rdiag + 1) * P if rdiag >= 0 else 0
                    nw = QT - c0
                    ss = ps_s.tile([P, 2 * QT], F32, tag="s")
                    pe = p_pool.tile([P, 2 * QT], BF16, tag="p", name="pe")
                    for h in range(HPC):
                        hp = ds(h * DH, DH)
                        nc.tensor.matmul(
                            ss[:, h * QT + c0 : (h + 1) * QT],
                            kt_sb[hp, ds(j * P, P)],
                            qt_sb[hp, ds(q4 * QT + c0, nw)],
                            start=True,
                            stop=True,
                        )
                    if c0 == 0:
                        exp_ranges(pe, ss, [(0, 2 * QT)])
                    else:
                        exp_ranges(
                            pe, ss,
                            [(h * QT + c0, nw) for h in range(HPC)],
                        )
                    if rdiag >= 0:
                        # keep iff qt >= kt  <=>  (col - p - 128*rdiag) >= 0
                        for h in range(HPC):
                            nc.gpsimd.affine_select(
                                out=pe[:, h * QT + c0 : h * QT + w_hi],
                                in_=pe[:, h * QT + c0 : h * QT + w_hi],
                                compare_op=OP.is_ge,
                                fill=0.0,
                                base=c0 - rdiag * P,
                                pattern=[[1, w_hi - c0]],
                                channel_multiplier=-1,
                            )
                    # PE filler between this chunk's scores and the
                    # (possibly exp-waiting) AV: finish this batch's own proj
                    # first, then feed the next batch's at a steady rate,
                    # leaving its last stages for attention(b+1) to carry.
                    if own_done < NQ4 * SOPS:
                        take = min(3, NQ4 * SOPS - own_done)
                        drain(g_own, take)
                        own_done += take
                    elif gnext is not None and next_done < NQ4 * SOPS:
                        take = min(3, NQ4 * SOPS - next_done)
                        drain(gnext, take)
                        next_done += take
                    pend.append((j, c0, pe))
                    # software-pipeline: AV runs a few chunks behind scores
                    if len(pend) > 5 or last:
                        for jj, cc0, ppe in pend if last else [pend[0]]:
                            for h in range(HPC):
                                nc.tensor.matmul(
                                    pz[h][0:VA, cc0:QT],
                                    va[h][:, jj, :],
                                    ppe[:, h * QT + cc0 : (h + 1) * QT],
                                    start=(jj == 0),
                                    stop=(jj == njs - 1),
                                    skip_group_check=True,
                                )
                        pend = [] if last else pend[1:]

                for h in range(HPC):
                    bal_copy(
                        zt_sb[h][:, ds(q4 * QT, QT)], pz[h][0:VA, :], QT
                    )
                    # z^T (+ denominator row) raw; host divides+transposes
                    nc.sync.dma_start(
                        out_d[b, h, :, ds(q4 * QT, QT)],
                        zt_sb[h][:, ds(q4 * QT, QT)],
                    )
            # hand the next batch's partially-drained projections over; its
            # own attention finishes them via stage gates and filler.
            g_own = gnext
            own_done = next_done if gnext is not None else 0


def build_bass():
    nc = bacc.Bacc(None, target_bir_lowering=False)
    xT_d = nc.declare_dram_parameter("xT", [B, D, T], BF16, isOutput=False)
    x0p_d = nc.declare_dram_parameter(
        "x0p", [NQ4, KD, P, QT], BF16, isOutput=False
    )
    w_d = {
        name: nc.declare_dram_parameter(name, [P, KD * H2], BF16, isOutput=False)
        for name in ("wq", "wk", "wv")
    }
    out_d = nc.declare_dram_parameter(
        "out", [B, HPC, DH + 1, T], BF16, isOutput=True
    )
    with tile.TileContext(nc) as tc:
        _build(nc, tc, xT_d, x0p_d, w_d, out_d)
    nc.compile()
    return nc


_CACHE = {}


def _get_nc():
    if "nc" not in _CACHE:
        _CACHE["nc"] = build_bass()
    return _CACHE["nc"]


def make_in_maps(x, W_K, W_Q, W_V):
    x = np.asarray(x, dtype=np.float32)
    xT = np.ascontiguousarray(np.transpose(x, (0, 2, 1))).astype(ml_dtypes.bfloat16)
    # batch-0 x^T prechunked into contiguous [t4, k, 128, 512] pieces
    x0p = np.ascontiguousarray(
        xT[0].reshape(KD, P, NQ4, QT).transpose(2, 0, 1, 3)
    )
    in_maps = []
    for c in range(NCORES):
        sl = slice(c * HPC, (c + 1) * HPC)

        def wt(w):
            w = np.asarray(w, dtype=np.float32)
            wt_ = w[sl].reshape(H2, D).T  # [D, H2]
            wt_ = wt_.reshape(KD, P, H2).transpose(1, 0, 2).reshape(P, KD * H2)
            return np.ascontiguousarray(wt_).astype(ml_dtypes.bfloat16)

        in_maps.append(
            {"xT": xT, "x0p": x0p, "wq": wt(W_Q), "wk": wt(W_K), "wv": wt(W_V)}
        )
    return in_maps


def kernel(x, W_K, W_Q, W_V, _trace=False, _trace_kwargs=None):
    in_maps = make_in_maps(x, W_K, W_Q, W_V)
    res = run_bass_kernel_spmd(
        _get_nc(),
        in_maps,
        list(range(NCORES)),
        trace=_trace,
        **(_trace_kwargs or {}),
    )
    _CACHE["last_results"] = res
    outs = []
    for c in range(NCORES):
        # [B, HPC, DH+1, T] bf16: z^T rows + softmax denominator row
        zt = np.asarray(res.results[c]["out"]).astype(np.float32)
        z = zt[:, :, :DH, :] / zt[:, :, DH : DH + 1, :]
        outs.append(np.transpose(z, (0, 3, 1, 2)).reshape(B, T, H2))
    return np.concatenate(outs, axis=2)

